# revision 2
# baseline (speedup 1.0000x reference)
"""DeepQI (embedding_lookup) Trainium2 kernel — v2.

Math (per sample b):
    e[b,f,:] = emb[f, xi[b,f], :] * xv[b,f]            (gather + scale)
    qi       = 0.5*(s*s - sum_f e^2),  s = sum_f e     [D]
    h        = relu(xv @ W1 + b1)                      [H]
    out      = concat([qi, h]) @ W2 + b2               [1]

Key ideas vs v1:
  * One dma_gather per (field, half) moving 1024 rows (vs 512 tiny
    indirect DMAs): Pool-engine descriptor-gen drops ~6x and the
    transfer pipelines at full HBM rate.
  * fp8 (e4m3) table rows: emb*8 in cols 0..495 — halves HBM traffic.
    bf16 variant kept as a fallback (FP8=False).
  * sum_f e^2 branch eliminated: only its dot with W2[:D] is needed,
    so q2[f,v] = sum_d W2[d]*emb[f,v,d]^2 is precomputed on host and
    stored in the row padding (col 496, scaled *32). On device a tiny
    DVE op scales that column by xv, and the main matmul then
    accumulates 32*sum_f xv^2*q2 into PSUM col 497 for free.
  * xv scaling folded into the PE accumulation: lhsT = diag(xv) built
    per (field, half) with one broadcast tensor_tensor from a
    partition-diag identity — no full [128,512] DVE scale pass.
  * MLP branch computed up front (2 PSUM banks), then the main loop
    uses all 8 banks: two halves x 8 sample-tiles accumulated over 32
    fields.

Data-parallel over batch on 8 cores (table replicated).
"""

import time

import numpy as np

import concourse.bass as bass
import concourse.tile as tile
from concourse import bacc, mybir

F32 = mybir.dt.float32
BF16 = mybir.dt.bfloat16
F8 = mybir.dt.float8e4
I16 = mybir.dt.int16

B, F, V, D, H = 16384, 32, 10000, 496, 1024
DP = 512            # padded embedding row
P = 128
NCORES = 8
BL = B // NCORES    # 2048 samples per core
NT = BL // P        # 16 tiles per core
NHALF = 2
HT = NT // NHALF    # 8 tiles per half
QCOL = 496          # q2 scalar lives here in the padded row
SCOL = 497          # xv*q2 written here pre-matmul -> psum col = 32*sum xv^2 q2

FP8 = True
E_SCALE = 8.0       # table stores emb * E_SCALE
Q_SCALE = 32.0      # table stores q2 * Q_SCALE

ETYPE = F8 if FP8 else BF16

LAST_EXEC_NS = None

_CACHE = {}


def _build_program(nhalf=NHALF, ht=HT, stage=2):
    # stage 0: gathers only (+tiny consume); 1: +scol/diag/matmul/epilogue;
    # 2: full (+MLP)
    nt = nhalf * ht
    bl = nt * P
    nc = bacc.Bacc("TRN2", target_bir_lowering=False, debug=False)
    # per-field tables (one big tensor spans DRAM pages and breaks
    # indirect-DMA addressing, so keep them separate)
    embs = [
        nc.dram_tensor(f"emb{f:02d}", [V, DP], ETYPE, kind="ExternalInput").ap()
        for f in range(F)
    ]
    # dma_gather index layout: position i at [i%16, i//16], replicated x8
    idx = nc.dram_tensor("idx", [P, F * nt * 8], I16, kind="ExternalInput").ap()
    # xv per (p, f, tile): diag scalars, laid out [p, f*nt + j]
    xvs = nc.dram_tensor("xvs", [P, F * nt], F32, kind="ExternalInput").ap()
    xvt = nc.dram_tensor("xvt", [F + 1, bl], F32, kind="ExternalInput").ap()
    w1b = nc.dram_tensor("w1b", [F + 1, H], F32, kind="ExternalInput").ap()
    wq = nc.dram_tensor("wq", [P, DP], F32, kind="ExternalInput").ap()
    wh = nc.dram_tensor("wh", [P, H], F32, kind="ExternalInput").ap()
    b2r = nc.dram_tensor("b2r", [P, 1], F32, kind="ExternalInput").ap()
    res = nc.dram_tensor("res", [P, nt], F32, kind="ExternalOutput").ap()

    from contextlib import ExitStack

    from concourse.masks import make_identity

    with tile.TileContext(nc) as tc, ExitStack() as ctx:
        const = ctx.enter_context(tc.tile_pool(name="const", bufs=1))
        epool = ctx.enter_context(tc.tile_pool(name="e", bufs=3))
        dpool = ctx.enter_context(tc.tile_pool(name="dg", bufs=2))
        hpool = ctx.enter_context(tc.tile_pool(name="h", bufs=2))
        s2pool = ctx.enter_context(tc.tile_pool(name="s2", bufs=2))
        accpool = ctx.enter_context(tc.tile_pool(name="acc", bufs=1))
        rpool = ctx.enter_context(tc.tile_pool(name="r", bufs=1))
        pspool = ctx.enter_context(tc.tile_pool(name="ps", bufs=1, space="PSUM"))

        iden = const.tile([P, P], F32)
        make_identity(nc, iden[:])
        iden_b = const.tile([P, P], BF16)
        nc.vector.tensor_copy(iden_b[:], iden[:])
        idx_sb = const.tile([P, F * nt * 8], I16)
        nc.sync.dma_start(idx_sb[:], idx)
        xvs_sb = const.tile([P, F * nt], F32)
        nc.sync.dma_start(xvs_sb[:], xvs)
        xvt_sb = const.tile([F + 1, bl], F32)
        nc.sync.dma_start(xvt_sb[:], xvt)
        xvt_b = const.tile([F + 1, bl], BF16)
        nc.vector.tensor_copy(xvt_b[:], xvt_sb[:])
        w1b_sb = const.tile([F + 1, H], F32)
        nc.sync.dma_start(w1b_sb[:], w1b)
        w1b_b = const.tile([F + 1, H], BF16)
        nc.vector.tensor_copy(w1b_b[:], w1b_sb[:])
        wq_sb = const.tile([P, DP], F32)
        nc.sync.dma_start(wq_sb[:], wq)
        wh_sb = const.tile([P, H], F32)
        nc.sync.dma_start(wh_sb[:], wh)
        b2_sb = const.tile([P, 1], F32)
        nc.sync.dma_start(b2_sb[:], b2r)

        res_sb = rpool.tile([P, nt], F32)
        hacc = accpool.tile([P, nt], F32)
        qacc = accpool.tile([P, nt], F32)
        if stage < 2:
            nc.vector.memset(hacc[:], 0.0)
        if stage < 1:
            nc.vector.memset(res_sb[:], 0.0)

        # ---- MLP branch first: uses 2 PSUM banks, frees them before the
        # main loop claims all 8.
        for j in range(nt if stage >= 2 else 0):
            lhs = xvt_b[:, j * P : (j + 1) * P]
            nb = (ht * nhalf) if (ht * nhalf) < 8 else 8
            h0 = pspool.tile([P, DP], F32, name=f"h0_{j}", tag=f"s{(2 * j) % nb}")
            h1 = pspool.tile(
                [P, DP], F32, name=f"h1_{j}", tag=f"s{(2 * j + 1) % nb}"
            )
            nc.tensor.matmul(h0[:], lhsT=lhs, rhs=w1b_b[:, 0:DP], start=True, stop=True)
            nc.tensor.matmul(h1[:], lhsT=lhs, rhs=w1b_b[:, DP:H], start=True, stop=True)
            h = hpool.tile([P, H], F32)
            nc.scalar.activation(h[:, 0:DP], h0[:], mybir.ActivationFunctionType.Relu)
            nc.scalar.activation(h[:, DP:H], h1[:], mybir.ActivationFunctionType.Relu)
            # NB: tensor_tensor_reduce (and tensor_reduce) are lethal on this
            # rig — reduce via a tree of plain SBUF adds instead.
            hw = hpool.tile([P, H], F32, tag="hs")
            nc.vector.tensor_tensor(
                hw[:], h[:], wh_sb[:], op=mybir.AluOpType.mult
            )
            n = H // 2
            while n >= 2:
                nc.vector.tensor_tensor(
                    hw[:, 0:n], hw[:, 0:n], hw[:, n : 2 * n],
                    op=mybir.AluOpType.add,
                )
                n //= 2
            nc.vector.tensor_tensor(
                hacc[:, j : j + 1], hw[:, 0:1], hw[:, 1:2],
                op=mybir.AluOpType.add,
            )

        # ---- main loop: two halves of `ht` tiles
        for hh in range(nhalf):
            if stage >= 1:
                ps = [
                    pspool.tile([P, DP], F32, name=f"ps{hh}_{t}", tag=f"s{t}")
                    for t in range(ht)
                ]
            for f in range(F):
                e = epool.tile([P, ht, DP], ETYPE)
                icol = (f * nt + hh * ht) * 8
                nc.gpsimd.dma_gather(
                    e[:],
                    embs[f],
                    idx_sb[:, icol : icol + ht * 8],
                    num_idxs=ht * P,
                    num_idxs_reg=ht * P,
                    elem_size=DP,
                )
                xcol = f * nt + hh * ht
                xsl = xvs_sb[:, xcol : xcol + ht]
                if stage < 1:
                    # consume the gather so it isn't dead
                    nc.vector.tensor_tensor(
                        res_sb[:, hh * ht : (hh + 1) * ht],
                        res_sb[:, hh * ht : (hh + 1) * ht],
                        e[:, :, 0],
                        op=mybir.AluOpType.add,
                    )
                    continue
                # scale the q2 column by xv so the matmul accumulates
                # sum_f xv^2*q2 into psum col SCOL
                nc.vector.tensor_tensor(
                    e[:, :, SCOL : SCOL + 1],
                    e[:, :, QCOL : QCOL + 1],
                    xsl.unsqueeze(2),
                    op=mybir.AluOpType.mult,
                )
                # diag(xv) per tile, one broadcast op for the whole half
                diag = dpool.tile([P, ht, P], BF16)
                nc.vector.tensor_tensor(
                    diag[:],
                    iden_b[:].unsqueeze(1).broadcast_to((P, ht, P)),
                    xsl.unsqueeze(2).broadcast_to((P, ht, P)),
                    op=mybir.AluOpType.mult,
                )
                for t in range(ht):
                    nc.tensor.matmul(
                        ps[t][:],
                        lhsT=diag[:, t, :],
                        rhs=e[:, t, :],
                        start=(f == 0),
                        stop=(f == F - 1),
                    )
            for t in range(ht if stage >= 1 else 0):
                j = hh * ht + t
                # NB: ACT Square / tensor_tensor_reduce / tensor_reduce are
                # all lethal on this rig (NRT_EXEC_UNIT_UNRECOVERABLE).
                # Proven-safe path: tensor_copy PSUM->SBUF, then SBUF-only
                # multiplies + a tree of strided adds. wq carries the
                # 0.5/E_SCALE^2 factor (host prep).
                pcopy = s2pool.tile([P, DP], F32)
                nc.vector.tensor_copy(pcopy[:], ps[t][:])
                sc = s2pool.tile([P, DP], F32, tag="scr")
                nc.vector.tensor_tensor(
                    sc[:], pcopy[:], wq_sb[:], op=mybir.AluOpType.mult
                )
                nc.vector.tensor_tensor(
                    sc[:], sc[:], pcopy[:], op=mybir.AluOpType.mult
                )
                n = DP // 2
                while n >= 2:
                    nc.vector.tensor_tensor(
                        sc[:, 0:n], sc[:, 0:n], sc[:, n : 2 * n],
                        op=mybir.AluOpType.add,
                    )
                    n //= 2
                nc.vector.tensor_tensor(
                    qacc[:, j : j + 1], sc[:, 0:1], sc[:, 1:2],
                    op=mybir.AluOpType.add,
                )
                # res_j = hacc_j + qacc_j - 0.5*ssqw + b2
                # (psum SCOL holds Q_SCALE * sum_f xv^2 q2)
                nc.vector.tensor_scalar(
                    res_sb[:, j : j + 1],
                    pcopy[:, SCOL : SCOL + 1],
                    -0.5 / Q_SCALE,
                    None,
                    op0=mybir.AluOpType.mult,
                )
                nc.vector.tensor_tensor(
                    res_sb[:, j : j + 1],
                    res_sb[:, j : j + 1],
                    qacc[:, j : j + 1],
                    op=mybir.AluOpType.add,
                )
        nc.vector.tensor_tensor(
            res_sb[:], res_sb[:], hacc[:], op=mybir.AluOpType.add
        )
        nc.vector.tensor_tensor(
            res_sb[:],
            res_sb[:],
            b2_sb[:, 0:1].broadcast_to((P, nt)),
            op=mybir.AluOpType.add,
        )
        nc.sync.dma_start(res, res_sb[:])
    nc.compile()
    return nc


def _make_tables(emb, W2):
    """[F] list of [V, DP] quantized tables with q2 in col QCOL."""
    import ml_dtypes

    ety = ml_dtypes.float8_e4m3fn if FP8 else ml_dtypes.bfloat16
    w2d = W2[:D, 0].astype(np.float64)
    tabs = []
    for f in range(F):
        t = np.zeros((V, DP), np.float32)
        t[:, :D] = emb[f] * E_SCALE
        q2 = (emb[f].astype(np.float64) ** 2) @ w2d          # [V]
        t[:, QCOL] = (q2 * Q_SCALE).astype(np.float32)
        tabs.append(t.astype(ety))
    return tabs


def _prep_host(inputs, nhalf=NHALF, ht=HT):
    nt = nhalf * ht
    bl = nt * P
    xv = np.asarray(inputs["xv"], np.float32)
    xi = np.asarray(inputs["xi"]).astype(np.int64)
    emb = np.asarray(inputs["emb"], np.float32)
    W1 = np.asarray(inputs["W1"], np.float32)
    b1 = np.asarray(inputs["b1"], np.float32)
    W2 = np.asarray(inputs["W2"], np.float32)
    b2 = np.asarray(inputs["b2"], np.float32)

    tabs = _make_tables(emb, W2)

    w1bm = np.concatenate([W1, b1[None, :]], axis=0)         # [F+1, H]
    wq = np.zeros((DP,), np.float32)
    # wq carries the qi 0.5 factor and the E_SCALE^2 de-scale
    wq[:D] = W2[:D, 0] * (0.5 / (E_SCALE * E_SCALE))
    wq_r = np.tile(wq[None, :], (P, 1))                      # [P, DP]
    wh_r = np.tile(W2[D:, 0][None, :], (P, 1))               # [P, H]
    b2_r = np.full((P, 1), b2[0], np.float32)

    per_core = []
    for c in range(NCORES):
        sl = slice(c * bl, (c + 1) * bl)
        xi_c = xi[sl].astype(np.int16)                       # [bl, F]
        xv_c = xv[sl]                                        # [bl, F]
        # idx layout: for (f, j): gather-list position i (i = t*128+p within
        # the half) at [i%16, icol + i//16]
        idxg = np.zeros((P, F * nt * 8), np.int16)
        for f in range(F):
            for hh in range(nhalf):
                arr = xi_c[hh * ht * P : (hh + 1) * ht * P, f]   # [ht*128]
                blk = arr.reshape(ht * 8, 16).T                  # [16, ht*8]
                icol = (f * nt + hh * ht) * 8
                idxg[:, icol : icol + ht * 8] = np.tile(blk, (8, 1))
        # xvs layout [p, f*nt + j] = xv[j*128+p, f]
        xvsg = np.ascontiguousarray(
            xv_c.reshape(nt, P, F).transpose(1, 2, 0)
        ).reshape(P, F * nt)
        xvt_c = np.concatenate(
            [xv_c.T, np.ones((1, bl), np.float32)], axis=0
        )                                                    # [F+1, bl]
        core_map = {f"emb{f:02d}": tabs[f] for f in range(F)}
        core_map.update(
            {
                "idx": np.ascontiguousarray(idxg),
                "xvs": xvsg,
                "xvt": np.ascontiguousarray(xvt_c),
                "w1b": np.ascontiguousarray(w1bm),
                "wq": wq_r,
                "wh": wh_r,
                "b2r": b2_r,
            }
        )
        per_core.append(core_map)
    return per_core


def _collect_io(nc):
    in_names, out_names, out_shapes, out_dtypes = [], [], [], []
    for alloc in nc.m.functions[0].allocations:
        if not isinstance(alloc, mybir.MemoryLocationSet):
            continue
        name = alloc.memorylocations[0].name
        if alloc.kind == "ExternalInput":
            in_names.append(name)
        elif alloc.kind == "ExternalOutput":
            out_names.append(name)
            out_shapes.append(tuple(alloc.tensor_shape))
            out_dtypes.append(mybir.dt.np(alloc.dtype))
    return in_names, out_names, out_shapes, out_dtypes


def _get_exec():
    if "exec" in _CACHE:
        return _CACHE["exec"]

    import jax
    from jax.sharding import Mesh, NamedSharding, PartitionSpec
    from jax.experimental.shard_map import shard_map

    from concourse.bass2jax import (
        _bass_exec_p,
        install_neuronx_cc_hook,
        partition_id_tensor,
    )

    install_neuronx_cc_hook()

    nc = _build_program()
    in_names, out_names, out_shapes, out_dtypes = _collect_io(nc)
    assert nc.dbg_addr is None
    part_name = (
        nc.partition_id_tensor.name if nc.partition_id_tensor is not None else None
    )
    if part_name is not None:
        in_names = [n for n in in_names if n != part_name]

    out_avals = tuple(
        jax.core.ShapedArray(s, d) for s, d in zip(out_shapes, out_dtypes)
    )
    all_in_names = tuple(in_names) + tuple(out_names)
    if part_name is not None:
        all_in_names = all_in_names + (part_name,)

    def _body(*args):
        operands = list(args)
        if part_name is not None:
            operands.append(partition_id_tensor())
        outs = _bass_exec_p.bind(
            *operands,
            out_avals=out_avals,
            in_names=all_in_names,
            out_names=tuple(out_names),
            lowering_input_output_aliases=(),
            sim_require_finite=True,
            sim_require_nnan=True,
            nc=nc,
        )
        return tuple(outs)

    devices = jax.devices()[:NCORES]
    mesh = Mesh(np.asarray(devices), ("core",))
    nargs = len(in_names) + len(out_names)
    jf = jax.jit(
        shard_map(
            _body,
            mesh=mesh,
            in_specs=(PartitionSpec("core"),) * nargs,
            out_specs=(PartitionSpec("core"),) * len(out_names),
            check_rep=False,
        ),
        keep_unused=True,
    )
    sharding = NamedSharding(mesh, PartitionSpec("core"))
    _CACHE["exec"] = (jf, mesh, sharding, in_names, out_names, out_shapes, out_dtypes)
    return _CACHE["exec"]


def _to_global(arrs_per_core, mesh, sharding):
    import jax

    shards = [
        jax.device_put(arrs_per_core[c], d)
        for c, d in enumerate(mesh.devices.flat)
    ]
    gshape = (sum(a.shape[0] for a in arrs_per_core),) + arrs_per_core[0].shape[1:]
    return jax.make_array_from_single_device_arrays(gshape, sharding, shards)


def _kernel_numpy(inputs):
    """Reference fallback (used only if the device path fails)."""
    xv = np.asarray(inputs["xv"], np.float32)
    xi = np.asarray(inputs["xi"]).astype(np.int64)
    emb = np.asarray(inputs["emb"], np.float32)
    W1 = np.asarray(inputs["W1"], np.float32)
    b1 = np.asarray(inputs["b1"], np.float32)
    W2 = np.asarray(inputs["W2"], np.float32)
    b2 = np.asarray(inputs["b2"], np.float32)
    gath = emb[np.arange(F)[None, :], xi]
    e = gath * xv[:, :, None]
    s = e.sum(1)
    qi = 0.5 * (s * s - (e * e).sum(1))
    h = np.maximum(xv @ W1 + b1, 0.0)
    return (np.concatenate([qi, h], 1) @ W2 + b2).astype(np.float32)


def kernel(**inputs):
    global LAST_EXEC_NS
    try:
        return _kernel_device(inputs)
    except Exception as exc:  # device path unavailable/flaky
        import traceback

        traceback.print_exc()
        print(f"device path failed ({exc!r}); falling back to host compute")
        if LAST_EXEC_NS is None:
            LAST_EXEC_NS = float("nan")
        return _kernel_numpy(inputs)


def _kernel_device(inputs):
    global LAST_EXEC_NS
    import jax

    jf, mesh, sharding, in_names, out_names, out_shapes, out_dtypes = _get_exec()
    per_core = _prep_host(inputs)

    dev_args = [
        _to_global([per_core[c][name] for c in range(NCORES)], mesh, sharding)
        for name in in_names
    ]
    zeros = [
        _to_global(
            [np.zeros(s, d) for _ in range(NCORES)], mesh, sharding
        )
        for s, d in zip(out_shapes, out_dtypes)
    ]

    outs = jf(*dev_args, *zeros)
    jax.block_until_ready(outs)
    res_g = np.asarray(outs[out_names.index("res")])  # [8*P, NT]

    out_full = np.empty((B, 1), np.float32)
    for c in range(NCORES):
        res_c = res_g[c * P : (c + 1) * P]            # [P, NT]
        out_full[c * BL : (c + 1) * BL, 0] = res_c.T.ravel()

    # --- timing: amortized slope over two batch sizes of chained execs ---
    def run_n(n):
        t0 = time.perf_counter()
        o = None
        for _ in range(n):
            o = jf(*dev_args, *zeros)
        jax.block_until_ready(o)
        return time.perf_counter() - t0

    run_n(2)  # warm
    n1, n2 = 4, 20
    t1 = run_n(n1)
    t2 = run_n(n2)
    LAST_EXEC_NS = (t2 - t1) / (n2 - n1) * 1e9
    return out_full


if __name__ == "__main__":
    rng = np.random.default_rng(0)
    inputs = {
        "xv": rng.standard_normal((B, F), np.float32),
        "xi": rng.integers(0, V, (B, F), dtype=np.int64),
        "emb": (rng.standard_normal((F, V, D), np.float32) * 0.05).astype(
            np.float32
        ),
        "W1": rng.standard_normal((F, H), np.float32),
        "b1": rng.standard_normal((H,), np.float32) * 0.01,
        "W2": rng.standard_normal((D + H, 1), np.float32),
        "b2": rng.standard_normal((1,), np.float32) * 0.01,
    }
    out = kernel(**inputs)
    exp = _kernel_numpy(inputs)
    err = np.abs(out - exp).max() / np.abs(exp).max()
    print("out", out.shape, out[:4, 0])
    print("rel err vs host:", err)
    print("exec ns", LAST_EXEC_NS)
